# revision 12
# baseline (speedup 1.0000x reference)
"""Multi-head attention block (B=4, N=2048, C=768, H=12) on 8 trn2 cores.

Sharding: core c handles batch b=c//2 and heads h0=6*(c%2)..h0+5
(data parallel on B x tensor parallel on heads). Each core computes its
6 heads' qkv projection, attention, and a partial output projection;
the host sums the two per-batch partials and adds proj_b.

All layout transforms (transposes / head permutations) happen on the
host so the device only runs dense matmuls:
  xT      [768, 2048]  x[b]^T (contraction dim c on partitions)
  wqk     [768, 768]   qkv_w rows for this core's q+k, transposed; cols
                       grouped per head-pair as [k_A|k_B | q_A|q_B]*3
  bias_qk [128, 6]     matching biases, one col per (pair, k/q) block
  wv      [768, 384]   v weights transposed, heads in order h0..h0+5
  wv_b    [1, 384]     v bias row
  wp      [384, 768]   proj_w cols for this core's heads, transposed,
                       rows grouped per pair [hA dims | hB dims]
Output: outT [768, 2048] = (partial out[b])^T, summed on host.

Attention per head pair (A,B stacked on partitions 0-63 / 64-127):
  S^T[m,n] = kT^T qT via row-tiled (tile_position 0/64) K=64 matmuls,
  P = exp(S/8) on ScalarE straight out of PSUM,
  out2[d,n] = v_aug^T P with a ones column giving the softmax sums in
  row 64; normalize with reciprocal + K=1 broadcast matmul.
"""

import sys

if "/opt/trn_rl_repo" not in sys.path:
    sys.path.insert(0, "/opt/trn_rl_repo")

import numpy as np

B, N, C = 4, 2048, 768
H, D = 12, 64
NCORES = 8
HPC = 6      # heads per core
NPAIRS = 3   # head pairs per core
NB = N // 512   # 512-wide n blocks
MT = N // 128   # 128-row m tiles
CT = C // 128   # c tiles

_CACHE = {}


def _build():
    from concourse import bacc
    import concourse.mybir as mybir
    from concourse.tile import TileContext

    F32 = mybir.dt.float32
    R = mybir.dt.float32r
    Exp = mybir.ActivationFunctionType.Exp

    nc = bacc.Bacc("TRN2", target_bir_lowering=False)
    xT = nc.dram_tensor("xT", [C, N], R, kind="ExternalInput")
    wqk = nc.dram_tensor("wqk", [C, 2 * HPC * D], R, kind="ExternalInput")
    bias_qk = nc.dram_tensor("bias_qk", [128, 2 * NPAIRS], F32, kind="ExternalInput")
    wv = nc.dram_tensor("wv", [C, HPC * D], R, kind="ExternalInput")
    wv_b = nc.dram_tensor("wv_b", [1, HPC * D], R, kind="ExternalInput")
    wp = nc.dram_tensor("wp", [HPC * D, C], R, kind="ExternalInput")
    outT = nc.dram_tensor("outT", [C, N], F32, kind="ExternalOutput")

    with nc.allow_low_precision(reason="float32r operands for full-rate matmul"), \
            TileContext(nc) as tc:
        with (
            tc.tile_pool(name="w", bufs=1) as w,
            tc.tile_pool(name="vp", bufs=1) as vp,
            tc.tile_pool(name="ap", bufs=1) as apool,
            tc.tile_pool(name="qk", bufs=2) as qk,
            tc.tile_pool(name="pt", bufs=4) as ptp,
            tc.tile_pool(name="small", bufs=2) as small,
            tc.tile_pool(name="ob", bufs=2) as ob,
            tc.tile_pool(name="mm", bufs=2, space="PSUM") as mmp,
            tc.tile_pool(name="sps", bufs=2, space="PSUM") as sps,
            tc.tile_pool(name="o2ps", bufs=1, space="PSUM") as o2ps,
        ):
            # ---- persistent loads, spread across DMA-issuing engines ----
            xt = [w.tile([128, N], R, tag=f"xt{i}", name=f"xt{i}")
                  for i in range(CT)]
            for nb in range(NB):
                nbs = slice(nb * 512, (nb + 1) * 512)
                for i in range(CT):
                    nc.sync.dma_start(xt[i][:, nbs], xT[i * 128:(i + 1) * 128, nbs])
            wqk_t = []
            for i in range(CT):
                t = w.tile([128, 2 * HPC * D], R, tag=f"wqk{i}", name=f"wqk{i}")
                nc.scalar.dma_start(t[:], wqk[i * 128:(i + 1) * 128, :])
                wqk_t.append(t)
            wv_t = []
            for i in range(CT):
                t = w.tile([128, HPC * D], R, tag=f"wv{i}", name=f"wv{i}")
                nc.gpsimd.dma_start(t[:], wv[i * 128:(i + 1) * 128, :])
                wv_t.append(t)
            wp_t = []
            for p in range(NPAIRS):
                t = w.tile([128, C], R, tag=f"wp{p}", name=f"wp{p}")
                nc.gpsimd.dma_start(t[:], wp[p * 128:(p + 1) * 128, :])
                wp_t.append(t)
            bqk = w.tile([128, 2 * NPAIRS], F32, tag="bqk")
            nc.gpsimd.dma_start(bqk[:], bias_qk[:])
            wvb = w.tile([1, HPC * D], R, tag="wvb")
            nc.gpsimd.dma_start(wvb[:], wv_b[:])
            onesf = w.tile([128, 128], F32, tag="onesf")
            nc.vector.memset(onesf[:], 1.0)
            ones128 = w.tile([1, 128], R, tag="ones128")
            nc.vector.tensor_copy(ones128[:], onesf[0:1, :])
            onescol = w.tile([128, HPC], R, tag="onescol")
            nc.vector.tensor_copy(onescol[:], onesf[:, 0:HPC])

            # v bias broadcast tile [128, HPC*D] via K=1 matmul
            psb = mmp.tile([128, HPC * D], F32, tag="mm")
            nc.tensor.matmul(psb[:], ones128[:], wvb[:], start=True, stop=True)
            vbias = w.tile([128, HPC, D], F32, tag="vbias")
            nc.vector.tensor_copy(vbias[:], psb.rearrange("p (h e) -> p h e", e=D))

            v_sb = [None] * MT

            def emit_v_m(m):
                ps = mmp.tile([128, HPC * D], F32, tag="mm", name="psv")
                for k in range(CT):
                    nc.tensor.matmul(ps[:], xt[k][:, m * 128:(m + 1) * 128],
                                     wv_t[k][:],
                                     start=(k == 0), stop=(k == CT - 1))
                vt = vp.tile([128, HPC, D + 1], R, tag=f"v{m}", name=f"v{m}")
                nc.vector.tensor_add(vt[:, :, 0:D],
                                     ps.rearrange("p (h e) -> p h e", e=D), vbias[:])
                nc.vector.tensor_copy(vt[:, :, D:D + 1], onescol[:])
                v_sb[m] = vt

            A_sb = [apool.tile([128, N], R, tag=f"A{p}", name=f"A{p}") for p in range(NPAIRS)]

            def emit_qk_half(p, nb, idx, tiles):
                """One 512-col block of pair p's k (idx even) or q (idx odd)."""
                kT, qT = tiles
                dest = kT if idx % 2 == 0 else qT
                co = (2 * p + idx % 2) * 128
                nbs = slice(nb * 512, (nb + 1) * 512)
                ps = mmp.tile([128, 512], F32, tag="mm", name="psqk")
                for k in range(CT):
                    nc.tensor.matmul(ps[:], wqk_t[k][:, co:co + 128],
                                     xt[k][:, nbs],
                                     start=(k == 0), stop=(k == CT - 1))
                nc.vector.tensor_scalar(out=dest[:, nbs], in0=ps[:],
                                        scalar1=bqk[:, 2 * p + idx % 2:2 * p + idx % 2 + 1],
                                        scalar2=None,
                                        op0=mybir.AluOpType.add)

            def alloc_qk(p):
                return (qk.tile([128, N], R, tag="kT", name=f"kT{p}"),
                        qk.tile([128, N], R, tag="qT", name=f"qT{p}"))

            def emit_proj_ct(nbs, ct):
                psp = mmp.tile([128, 512], F32, tag="mm", name="psp")
                for pp in range(NPAIRS):
                    nc.tensor.matmul(psp[:], wp_t[pp][:, ct * 128:(ct + 1) * 128],
                                     A_sb[pp][:, nbs],
                                     start=(pp == 0), stop=(pp == NPAIRS - 1))
                osb = ob.tile([128, 512], F32, tag="osb", name="osb")
                nc.vector.tensor_copy(osb[:], psp[:])
                nc.sync.dma_start(outT[ct * 128:(ct + 1) * 128, nbs], osb[:])

            # ---- attention: flat software pipeline over (pair, nb, m) ----
            # S/exp run 2 units ahead of attn@v so the ScalarE exp stream
            # never stalls at n-block boundaries. All fill-in PE work (v
            # projection, qk projections, output projection) is queued as
            # small deadline-tagged items drained ~one per unit into the PE
            # slack behind the exp stream.
            import collections as _c
            pair_tiles = {0: alloc_qk(0)}
            pts = {}
            o2s = {}
            prework = _c.deque()

            def emit_S(p, nb, m):
                kT, qT = pair_tiles[p]
                nbs = slice(nb * 512, (nb + 1) * 512)
                msl = slice(m * 128, (m + 1) * 128)
                st = sps.tile([128, 2, 512], F32, tag="s", name="st")
                nc.tensor.matmul(st[:, 0], kT[0:64, msl], qT[0:64, nbs],
                                 start=True, stop=True, tile_position=(0, 0))
                nc.tensor.matmul(st[:, 1], kT[64:128, msl], qT[64:128, nbs],
                                 start=True, stop=True, tile_position=(64, 0))
                pt = ptp.tile([128, 2, 512], R, tag="pt", name="pt")
                nc.scalar.activation(pt[:], st[:], Exp, scale=float(D) ** -0.5)
                pts[(p, nb, m)] = pt

            def emit_av(p, nb, m):
                if m == 0:
                    o2s[(p, nb)] = (
                        o2ps.tile([D + 1, 512], F32, tag="o2a", name="o2a"),
                        o2ps.tile([D + 1, 512], F32, tag="o2b", name="o2b"))
                o2a, o2b = o2s[(p, nb)]
                pt = pts.pop((p, nb, m))
                first, last = m == 0, m == MT - 1
                nc.tensor.matmul(o2a[:], v_sb[m][:, 2 * p, :], pt[:, 0],
                                 start=first, stop=last)
                nc.tensor.matmul(o2b[:], v_sb[m][:, 2 * p + 1, :], pt[:, 1],
                                 start=first, stop=last)

            def emit_normalize(p, nb, i):
                nbs = slice(nb * 512, (nb + 1) * 512)
                o2a, o2b = o2s.pop((p, nb))
                for hh, o2 in ((0, o2a), (1, o2b)):
                    rec = small.tile([1, 512], R, tag="rec", name="rec")
                    nc.vector.reciprocal(rec[:], o2[D:D + 1, :])
                    bc = mmp.tile([64, 512], F32, tag="mm", name="bc")
                    nc.tensor.matmul(bc[:], ones128[:, 0:64], rec[:],
                                     start=True, stop=True)
                    bcs = small.tile([64, 512], F32, tag="bcs", name="bcs")
                    nc.vector.tensor_copy(bcs[:], bc[:])
                    nc.vector.tensor_mul(A_sb[p][hh * 64:(hh + 1) * 64, nbs],
                                         o2[0:D, :], bcs[:])
                if p + 1 < NPAIRS:
                    # prefetch next pair's k/q blocks for this nb
                    kd = 64 * (p + 1) + 4 * nb    # k block nb feeds m-tiles 4nb..
                    qd = 64 * (p + 1) + 16 * nb   # q block nb feeds its n-block
                    prework.append((min(kd, i + 2), lambda p=p, nb=nb:
                                    emit_qk_half(p + 1, nb, 0, pair_tiles[p + 1])))
                    prework.append((min(qd, i + 3), lambda p=p, nb=nb:
                                    emit_qk_half(p + 1, nb, 1, pair_tiles[p + 1])))
                if p == NPAIRS - 1:
                    for ct in range(CT):
                        prework.append((i + 2 + ct,
                                        lambda nbs=nbs, ct=ct: emit_proj_ct(nbs, ct)))

            # seed: pair-0 nb-0 projections immediately, the rest just in time
            seed = [(0, lambda: emit_qk_half(0, 0, 0, pair_tiles[0])),
                    (0, lambda: emit_qk_half(0, 0, 1, pair_tiles[0]))]
            for nbp in range(1, NB):
                seed.append((4 * nbp, lambda nbp=nbp:
                             emit_qk_half(0, nbp, 0, pair_tiles[0])))
                seed.append((16 * nbp - 2, lambda nbp=nbp:
                             emit_qk_half(0, nbp, 1, pair_tiles[0])))
            for m in range(MT):
                seed.append((m + 1, lambda m=m: emit_v_m(m)))
            seed.sort(key=lambda x: x[0])
            prework.extend(seed)

            units = [(p, nb, m) for p in range(NPAIRS) for nb in range(NB)
                     for m in range(MT)]
            for i, (p, nb, m) in enumerate(units):
                if nb == 0 and m == 0 and p + 1 < NPAIRS:
                    pair_tiles[p + 1] = alloc_qk(p + 1)
                emitted = 0
                while prework and (prework[0][0] <= i or emitted == 0):
                    if prework[0][0] > i and emitted > 0:
                        break
                    _, fn = prework.popleft()
                    fn()
                    emitted += 1
                emit_S(p, nb, m)
                if i >= 2:
                    pp, pnb, pm = units[i - 2]
                    emit_av(pp, pnb, pm)
                    if pm == MT - 1:
                        emit_normalize(pp, pnb, i)
            for j, (pp, pnb, pm) in enumerate(units[-2:]):
                emit_av(pp, pnb, pm)
                if pm == MT - 1:
                    emit_normalize(pp, pnb, len(units) + j)
            while prework:
                _, fn = prework.popleft()
                fn()

    nc.compile()
    return nc


def make_in_maps(x, qkv_w, qkv_b, proj_w):
    """Host-side sharding: per-core input dicts."""
    x = np.asarray(x, np.float32)
    qkv_w = np.asarray(qkv_w, np.float32)
    qkv_b = np.asarray(qkv_b, np.float32)
    proj_w = np.asarray(proj_w, np.float32)
    in_maps = []
    for c in range(NCORES):
        b, h0 = c // 2, HPC * (c % 2)
        heads = list(range(h0, h0 + HPC))
        perm_qk, perm_v, perm_p = [], [], []
        for p in range(NPAIRS):
            hA, hB = h0 + 2 * p, h0 + 2 * p + 1
            perm_qk += list(range(C + hA * D, C + hA * D + D))   # k_A
            perm_qk += list(range(C + hB * D, C + hB * D + D))   # k_B
            perm_qk += list(range(hA * D, hA * D + D))           # q_A
            perm_qk += list(range(hB * D, hB * D + D))           # q_B
            perm_p += list(range(hA * D, hA * D + D))
            perm_p += list(range(hB * D, hB * D + D))
        for h in heads:
            perm_v += list(range(2 * C + h * D, 2 * C + h * D + D))
        in_maps.append({
            "xT": np.ascontiguousarray(x[b].T),
            "wqk": np.ascontiguousarray(qkv_w[perm_qk, :].T),
            "bias_qk": np.ascontiguousarray(
                qkv_b[np.array(perm_qk)].reshape(2 * NPAIRS, 128).T),
            "wv": np.ascontiguousarray(qkv_w[perm_v, :].T),
            "wv_b": qkv_b[np.array(perm_v)].reshape(1, HPC * D).copy(),
            "wp": np.ascontiguousarray(proj_w[:, perm_p].T),
        })
    return in_maps


def combine(results, proj_b):
    """Host-side unshard: sum per-batch partials, add bias, transpose."""
    proj_b = np.asarray(proj_b, np.float32)
    out = np.empty((B, N, C), np.float32)
    for b in range(B):
        acc = results[2 * b]["outT"] + results[2 * b + 1]["outT"]
        out[b] = acc.T + proj_b[None, :]
    return out


def kernel(x, qkv_w, qkv_b, proj_w, proj_b):
    from concourse.bass_utils import run_bass_kernel_spmd

    if "nc" not in _CACHE:
        _CACHE["nc"] = _build()
    nc = _CACHE["nc"]
    in_maps = make_in_maps(x, qkv_w, qkv_b, proj_w)
    res = run_bass_kernel_spmd(nc, in_maps, core_ids=list(range(NCORES)))
    return combine(res.results, proj_b)
